# revision 19
# baseline (speedup 1.0000x reference)
"""AttentionBlock kernel for 8 Trainium2 NeuronCores.

Reference op: GroupNorm(8 groups) -> 1x1 conv qkv -> 8-head attention over
1024 spatial positions -> 1x1 conv proj -> residual.   Shapes (full):
x [8, 512, 32, 32]; qkv_w [1536, 512]; proj_w [512, 512].

Sharding: pure data-parallel over batch - one batch element per core.

Per-core layout notes:
  - Channels live on partitions in 4 chunks of 128; spatial N=1024 on free.
  - GroupNorm cross-partition stats via a block-diagonal (1/64) mask matmul.
  - Scores are computed transposed (S^T = K^T Q, keys on partitions) so the
    softmax denominator is a matmul reduction and A'V needs no transposes.
  - V is produced transposed ([spatial, d]) directly by the qkv matmul with
    a ones column appended, so the A'V matmul's 65th output row is the
    softmax sum for free.
  - Matmuls use float32r (1 cycle/row for N>=256 vs 4 for plain fp32).
  - exp() has no max-subtraction: logits are ~N(0,1), |logit| < ~9, exp is
    safe in fp32 and softmax is shift-invariant.
"""

import os

import numpy as np
import ml_dtypes

NCORES = 8
C = 512
N = 1024  # 32*32 spatial
NH = 8
HD = 64  # head dim
CCH = 4  # channel chunks of 128
EPS = 1e-5

_CACHE = {}
LAST = {"exec_time_ns": None, "results": None}


def _build_program():
    import concourse.bass as bass
    import concourse.tile as tile
    from concourse import mybir

    f32 = mybir.dt.float32
    f32r = mybir.dt.float32r
    bf16 = mybir.dt.bfloat16
    AF = mybir.ActivationFunctionType
    OP = mybir.AluOpType

    nc = bass.Bass()

    # ---- DRAM parameters (per core). Host pre-reshapes/pre-transposes. ----
    x_d = nc.declare_dram_parameter("x", [CCH, 128, N], f32, isOutput=False)
    qkvw_d = nc.declare_dram_parameter("qkv_wT", [CCH, 128, 3 * C], bf16, isOutput=False)
    qb_d = nc.declare_dram_parameter("qb", [1, C], bf16, isOutput=False)
    kb_d = nc.declare_dram_parameter("kb", [1, C], bf16, isOutput=False)
    vb_d = nc.declare_dram_parameter("vb", [1, C], bf16, isOutput=False)
    pw_d = nc.declare_dram_parameter("proj_wT", [CCH, 128, C], bf16, isOutput=False)
    pb_d = nc.declare_dram_parameter("pb", [CCH, 128, 1], f32, isOutput=False)
    gnw_d = nc.declare_dram_parameter("gnw", [128, CCH], f32, isOutput=False)
    gnb_d = nc.declare_dram_parameter("gnb", [128, CCH], f32, isOutput=False)
    mask_d = nc.declare_dram_parameter("gn_mask", [128, 128], f32, isOutput=False)
    out_d = nc.declare_dram_parameter("out", [CCH, 128, N], f32, isOutput=True)

    def r(ap):
        return ap.bitcast(f32r)

    from contextlib import ExitStack

    with (
        nc.allow_low_precision(reason="float32r tiles feed full-speed matmuls"),
        tile.TileContext(nc) as tc,
        ExitStack() as ctx,
    ):
        consts = ctx.enter_context(tc.tile_pool(name="consts", bufs=1))
        xp = ctx.enter_context(tc.tile_pool(name="xp", bufs=1))
        qkp = ctx.enter_context(tc.tile_pool(name="qkp", bufs=1))
        vtp = ctx.enter_context(tc.tile_pool(name="vtp", bufs=1))
        ap_pool = ctx.enter_context(tc.tile_pool(name="ap", bufs=24))
        op_pool = ctx.enter_context(tc.tile_pool(name="op", bufs=1))
        misc = ctx.enter_context(tc.tile_pool(name="misc", bufs=2))
        gnp = ctx.enter_context(tc.tile_pool(name="gnp", bufs=1))
        outp = ctx.enter_context(tc.tile_pool(name="outp", bufs=2))
        ps_small = ctx.enter_context(tc.tile_pool(name="ps_s", bufs=3, space="PSUM"))
        ps_gn = ctx.enter_context(tc.tile_pool(name="ps_g", bufs=1, space="PSUM"))
        ps_big = ctx.enter_context(tc.tile_pool(name="ps_b", bufs=2, space="PSUM"))

        # ---- constants ----
        mask_sb = consts.tile([128, 128], f32, tag="mask")
        nc.sync.dma_start(out=mask_sb, in_=mask_d[:, :])
        vb_row = consts.tile([1, C], bf16, tag="vbrow")
        nc.sync.dma_start(out=vb_row, in_=vb_d[:, :])
        qb_row = consts.tile([1, C], bf16, tag="qbrow")
        nc.sync.dma_start(out=qb_row, in_=qb_d[:, :])
        kb_row = consts.tile([1, C], bf16, tag="kbrow")
        nc.sync.dma_start(out=kb_row, in_=kb_d[:, :])
        ones1 = consts.tile([1, 128], bf16, tag="ones1")
        nc.vector.memset(ones1, 1.0)
        ones_row = consts.tile([1, 512], bf16, tag="onesrow")
        nc.vector.memset(ones_row, 1.0)
        eps_sb = consts.tile([128, 1], f32, tag="eps")
        nc.vector.memset(eps_sb, EPS)
        zero_sb = consts.tile([128, 1], f32, tag="zero")
        nc.vector.memset(zero_sb, 0.0)
        pb_sb = []
        for cc in range(CCH):
            t = consts.tile([128, 1], f32, tag=f"pb{cc}")
            nc.sync.dma_start(out=t, in_=pb_d[cc])
            pb_sb.append(t)
        gnw_all = consts.tile([128, CCH], f32, tag="gnw")
        nc.sync.dma_start(out=gnw_all, in_=gnw_d[:, :])
        gnb_all = consts.tile([128, CCH], f32, tag="gnb")
        nc.sync.dma_start(out=gnb_all, in_=gnb_d[:, :])

        # ---- load x, weights ----
        x_sb = []
        for cc in range(CCH):
            t = xp.tile([128, N], f32, tag=f"x{cc}")
            nc.sync.dma_start(out=t, in_=x_d[cc])
            x_sb.append(t)

        with (
            tc.tile_pool(name="wq", bufs=1) as wq_pool,
            tc.tile_pool(name="xn", bufs=1) as xn_pool,
        ):
            qkvw_sb = []
            for cc in range(CCH):
                t = wq_pool.tile([128, 3 * C], bf16, tag=f"qw{cc}")
                nc.sync.dma_start(out=t, in_=qkvw_d[cc])
                qkvw_sb.append(t)

            # ---- GroupNorm (stats batched across the 4 channel chunks) ----
            mv_all = gnp.tile([128, CCH, 2], f32, tag="mv")
            for cc in range(CCH):
                stats = gnp.tile([128, 2, 6], f32, tag=f"st{cc}")
                for sg in range(2):
                    nc.vector.bn_stats(
                        out=stats[:, sg, :], in_=x_sb[cc][:, sg * 512 : (sg + 1) * 512]
                    )
                nc.vector.bn_aggr(out=mv_all[:, cc, :], in_=stats)
            # st2 = [mean_p, var_p + mean_p^2] for all chunks
            st2 = gnp.tile([128, CCH, 2], f32, tag="s2")
            nc.vector.tensor_copy(out=st2[:, :, 0], in_=mv_all[:, :, 0])
            mean_sq = gnp.tile([128, CCH], f32, tag="msq")
            nc.vector.tensor_mul(out=mean_sq, in0=mv_all[:, :, 0], in1=mv_all[:, :, 0])
            nc.vector.tensor_add(out=st2[:, :, 1], in0=mv_all[:, :, 1], in1=mean_sq)
            # group-average across the 64-channel blocks (exact fp32 matmul)
            ps_st = ps_gn.tile([128, CCH * 2], f32, tag="psgn")
            dep_nop = nc.tensor.nop(hint="dep").ins
            dep_nop.ins = [nc.tensor.lower_ap(mask_sb), nc.tensor.lower_ap(st2[:, :, :])]
            nc.tensor.matmul(
                ps_st,
                lhsT=mask_sb,
                rhs=st2.rearrange("p c two -> p (c two)"),
                start=True,
                stop=True,
            )
            gst_sb = gnp.tile([128, CCH, 2], f32, tag="gstsb")
            nc.vector.tensor_copy(
                out=gst_sb, in_=ps_st.rearrange("p (c two) -> p c two", two=2)
            )
            gst = gst_sb
            gm2 = gnp.tile([128, CCH], f32, tag="g2")
            nc.vector.tensor_mul(out=gm2, in0=gst[:, :, 0], in1=gst[:, :, 0])
            gvar = gnp.tile([128, CCH], f32, tag="gv")
            nc.vector.tensor_sub(out=gvar, in0=gst[:, :, 1], in1=gm2)
            # rstd = exp(-0.5*ln(var+eps)): stays in the same ACT table set as
            # the attention exp (natural_log_exp), avoiding a table switch.
            lnv = gnp.tile([128, CCH], f32, tag="lnv")
            nc.scalar.activation(out=lnv, in_=gvar, func=AF.Ln, bias=eps_sb)
            rstd = gnp.tile([128, CCH], f32, tag="rstd")
            nc.scalar.activation(out=rstd, in_=lnv, func=AF.Exp, scale=-0.5, bias=zero_sb)
            gscale = gnp.tile([128, CCH], f32, tag="gs")
            nc.vector.tensor_mul(out=gscale, in0=rstd, in1=gnw_all)
            t4 = gnp.tile([128, CCH], f32, tag="t4")
            nc.vector.tensor_mul(out=t4, in0=gst[:, :, 0], in1=gscale)
            gbias = gnp.tile([128, CCH], f32, tag="gb")
            nc.vector.tensor_sub(out=gbias, in0=gnb_all, in1=t4)
            xn_sb = []
            for cc in range(CCH):
                xn = xn_pool.tile([128, N], bf16, tag=f"xn{cc}")
                nc.vector.tensor_scalar(
                    out=xn,
                    in0=x_sb[cc],
                    scalar1=gscale[:, cc : cc + 1],
                    scalar2=gbias[:, cc : cc + 1],
                    op0=OP.mult,
                    op1=OP.add,
                )
                xn_sb.append(xn)

            # ---- qkv matmuls ----
            # vT first so attention pair 0 can start as early as possible.
            # vT layout: [m_chunk partitions, head, 65] (64 d + ones column)
            vt_sb = []
            for mi in range(8):
                vt = vtp.tile([128, NH, HD + 1], bf16, tag=f"vt{mi}")
                nc.vector.memset(vt[:, :, HD : HD + 1], 1.0)
                ps = ps_small.tile([128, C], f32, tag="ps")
                for cc in range(CCH):
                    nc.tensor.matmul(
                        ps,
                        lhsT=(xn_sb[cc][:, mi * 128 : (mi + 1) * 128]),
                        rhs=(qkvw_sb[cc][:, 2 * C : 3 * C]),
                        start=(cc == 0),
                        stop=False,
                    )
                nc.tensor.matmul(
                    ps,
                    lhsT=(ones1[0:1, 0:128]),
                    rhs=(vb_row),
                    start=False,
                    stop=True,
                )
                nc.scalar.copy(
                    out=vt[:, :, 0:HD],
                    in_=ps.rearrange("p (h d) -> p h d", h=NH),
                )
                vt_sb.append(vt)

            # q, k standard layout [o_chunk partitions, n free]; bias folded in
            # as a K=1 matmul; k also folds the 1/sqrt(hd) attention scale.
            q_sb = []
            k_sb = []
            for oc in range(CCH):
                for which, brow, dest in (("q", qb_row, q_sb), ("k", kb_row, k_sb)):
                    base = 0 if which == "q" else C
                    t = qkp.tile([128, N], bf16, tag=f"{which}{oc}")
                    for nj in range(2):
                        ps = ps_small.tile([128, 512], f32, tag="ps")
                        for cc in range(CCH):
                            nc.tensor.matmul(
                                ps,
                                lhsT=(qkvw_sb[cc][:, base + oc * 128 : base + (oc + 1) * 128]),
                                rhs=(xn_sb[cc][:, nj * 512 : (nj + 1) * 512]),
                                start=(cc == 0),
                                stop=False,
                            )
                        nc.tensor.matmul(
                            ps,
                            lhsT=(brow[0:1, oc * 128 : (oc + 1) * 128]),
                            rhs=(ones_row),
                            start=False,
                            stop=True,
                        )
                        tslice = t[:, nj * 512 : (nj + 1) * 512]
                        if which == "q":
                            nc.scalar.copy(out=tslice, in_=ps)
                        else:
                            nc.scalar.mul(out=tslice, in_=ps, mul=float(HD**-0.5))
                    dest.append(t)

        # ---- attention, head-pair by head-pair ----
        o_sb = []
        for p in range(CCH):
            o_tile = op_pool.tile([128, N], bf16, tag=f"o{p}")
            o_sb.append(o_tile)
        for p in range(CCH):  # head pair p = heads (2p, 2p+1); chunk p of q/k
            a_tiles = [[], []]
            for mi in range(8):
                for hh in range(2):
                    ps_e = ps_big.tile([128, 1024], f32, tag="pse")
                    for nj in range(2):
                        nc.tensor.matmul(
                            ps_e[:, nj * 512 : (nj + 1) * 512],
                            lhsT=(
                                k_sb[p][
                                    hh * 64 : (hh + 1) * 64, mi * 128 : (mi + 1) * 128
                                ]
                            ),
                            rhs=(
                                q_sb[p][
                                    hh * 64 : (hh + 1) * 64, nj * 512 : (nj + 1) * 512
                                ]
                            ),
                            start=True,
                            stop=True,
                        )
                    at = ap_pool.tile([128, 1024], bf16, tag="a")
                    nc.scalar.activation(out=at, in_=ps_e, func=AF.Exp, bias=zero_sb)
                    a_tiles[hh].append(at)
            for hh in range(2):
                h = 2 * p + hh
                for nj in range(2):
                    av = ps_small.tile([128, 512], f32, tag="ps")
                    for mi in range(8):
                        nc.tensor.matmul(
                            av[0 : HD + 1, :],
                            lhsT=vt_sb[mi][:, h, :],
                            rhs=a_tiles[hh][mi][:, nj * 512 : (nj + 1) * 512],
                            start=(mi == 0),
                            stop=(mi == 7),
                        )
                    rec = misc.tile([1, 512], bf16, tag="rec")
                    nc.vector.reciprocal(out=rec, in_=av[HD : HD + 1, :])
                    rep_ps = ps_small.tile([128, 512], f32, tag="ps")
                    nc.tensor.matmul(
                        rep_ps[0:64, :],
                        lhsT=(ones1[0:1, 0:64]),
                        rhs=(rec),
                        start=True,
                        stop=True,
                    )
                    rep_sb = misc.tile([64, 512], bf16, tag="rep")
                    nc.vector.tensor_copy(out=rep_sb, in_=rep_ps[0:64, :])
                    nc.vector.tensor_mul(
                        out=o_sb[p][hh * 64 : (hh + 1) * 64, nj * 512 : (nj + 1) * 512],
                        in0=av[0:HD, :],
                        in1=rep_sb,
                    )

        # ---- proj + residual ----
        pw_sb = []
        for cc in range(CCH):
            t = consts.tile([128, C], bf16, tag=f"pw{cc}")
            nc.sync.dma_start(out=t, in_=pw_d[cc])
            pw_sb.append(t)
        for oc in range(CCH):
            ot = outp.tile([128, N], f32, tag="ot")
            for nj in range(2):
                ps = ps_small.tile([128, 512], f32, tag="ps")
                for cc in range(CCH):
                    nc.tensor.matmul(
                        ps,
                        lhsT=(pw_sb[cc][:, oc * 128 : (oc + 1) * 128]),
                        rhs=(o_sb[cc][:, nj * 512 : (nj + 1) * 512]),
                        start=(cc == 0),
                        stop=(cc == CCH - 1),
                    )
                nc.vector.scalar_tensor_tensor(
                    out=ot[:, nj * 512 : (nj + 1) * 512],
                    in0=ps,
                    scalar=pb_sb[oc],
                    in1=x_sb[oc][:, nj * 512 : (nj + 1) * 512],
                    op0=OP.add,
                    op1=OP.add,
                )
            nc.sync.dma_start(out=out_d[oc], in_=ot)

    _split_lw_waits(nc)
    return nc


def _split_lw_waits(nc):
    """This walrus build accepts only ONE sync-wait command per engine
    instruction; Tile can attach several. Hoist each excess wait onto its own
    pure sem-wait instruction inserted just before, in queue order."""
    from concourse import mybir

    for blk in nc.m.functions[0].blocks:
        out = []
        for inst in blk.instructions:
            si = getattr(inst, "sync_info", None)
            if (
                si is not None
                and si.on_wait
                and len(si.on_wait) > 1
                and type(inst).__name__ != "InstEventSemaphore"
            ):
                waits = list(si.on_wait)
                for j, w in enumerate(waits[:-1]):
                    sem = mybir.InstEventSemaphore(
                        name=f"{inst.name}_wsplit{j}",
                        engine=inst.engine,
                        ins=[],
                        outs=[],
                        sync_info=mybir.SyncInfo(on_wait=[w], on_update=[]),
                    )
                    out.append(sem)
                inst.sync_info = mybir.SyncInfo(
                    on_wait=waits[-1:], on_update=list(si.on_update or [])
                )
            out.append(inst)
        blk.instructions = out


def kernel(x, gn_w, gn_b, qkv_w, qkv_b, proj_w, proj_b):
    from concourse.bass_utils import run_bass_kernel_spmd

    B = x.shape[0]
    assert B == NCORES
    if "nc" not in _CACHE:
        _CACHE["nc"] = _build_program()
    nc = _CACHE["nc"]

    xf = np.ascontiguousarray(x.reshape(B, CCH, 128, N).astype(np.float32))
    qkv_wT = np.ascontiguousarray(qkv_w.T.reshape(CCH, 128, 3 * C).astype(ml_dtypes.bfloat16))
    pw_T = np.ascontiguousarray(proj_w.T.reshape(CCH, 128, C).astype(ml_dtypes.bfloat16))
    qb = np.ascontiguousarray(qkv_b[0:C].reshape(1, C).astype(ml_dtypes.bfloat16))
    kb = np.ascontiguousarray(qkv_b[C : 2 * C].reshape(1, C).astype(ml_dtypes.bfloat16))
    vb = np.ascontiguousarray(qkv_b[2 * C : 3 * C].reshape(1, C).astype(ml_dtypes.bfloat16))
    pb = np.ascontiguousarray(proj_b.reshape(CCH, 128, 1).astype(np.float32))
    gnw = np.ascontiguousarray(gn_w.reshape(CCH, 128).T.astype(np.float32))
    gnb = np.ascontiguousarray(gn_b.reshape(CCH, 128).T.astype(np.float32))
    mask = np.zeros((128, 128), dtype=np.float32)
    for g in range(2):
        mask[g * 64 : (g + 1) * 64, g * 64 : (g + 1) * 64] = 1.0 / 64.0

    in_maps = []
    for i in range(NCORES):
        in_maps.append(
            {
                "x": xf[i],
                "qkv_wT": qkv_wT,
                "qb": qb,
                "kb": kb,
                "vb": vb,
                "proj_wT": pw_T,
                "pb": pb,
                "gnw": gnw,
                "gnb": gnb,
                "gn_mask": mask,
            }
        )

    tmpdir = os.environ.get("BASS_TMPDIR")
    if tmpdir:
        os.makedirs(tmpdir, exist_ok=True)
    res = run_bass_kernel_spmd(
        nc,
        in_maps,
        list(range(NCORES)),
        trace=bool(os.environ.get("BASS_TRACE")),
        tmpdir=tmpdir,
    )
    LAST["exec_time_ns"] = res.exec_time_ns
    LAST["results"] = res
    out = np.stack([res.results[i]["out"] for i in range(NCORES)], axis=0)
    return out.reshape(B, C, 32, 32).astype(x.dtype)


# revision 22
# speedup vs baseline: 1.4275x; 1.4275x over previous
"""AttentionBlock kernel for 8 Trainium2 NeuronCores.

Reference op: GroupNorm(8 groups) -> 1x1 conv qkv -> 8-head attention over
1024 spatial positions -> 1x1 conv proj -> residual.   Shapes (full):
x [8, 512, 32, 32]; qkv_w [1536, 512]; proj_w [512, 512].

Sharding: pure data-parallel over batch - one batch element per core.

Per-core layout notes:
  - Channels live on partitions in 4 chunks of 128; spatial N=1024 on free.
  - GroupNorm cross-partition stats via a block-diagonal (1/64) mask matmul.
  - Scores are computed transposed (S^T = K^T Q, keys on partitions) so the
    softmax denominator is a matmul reduction and A'V needs no transposes.
  - V is produced transposed ([spatial, d]) directly by the qkv matmul with
    a ones column appended, so the A'V matmul's 65th output row is the
    softmax sum for free.
  - Matmuls use float32r (1 cycle/row for N>=256 vs 4 for plain fp32).
  - exp() has no max-subtraction: logits are ~N(0,1), |logit| < ~9, exp is
    safe in fp32 and softmax is shift-invariant.
"""

import os

import numpy as np
import ml_dtypes

NCORES = 8
C = 512
N = 1024  # 32*32 spatial
NH = 8
HD = 64  # head dim
CCH = 4  # channel chunks of 128
EPS = 1e-5

_CACHE = {}
LAST = {"exec_time_ns": None, "results": None}


def _build_program():
    import concourse.bass as bass
    import concourse.tile as tile
    from concourse import mybir

    f32 = mybir.dt.float32
    f32r = mybir.dt.float32r
    bf16 = mybir.dt.bfloat16
    AF = mybir.ActivationFunctionType
    OP = mybir.AluOpType

    nc = bass.Bass()

    # ---- DRAM parameters (per core). Host pre-reshapes/pre-transposes. ----
    x_d = nc.declare_dram_parameter("x", [CCH, 128, N], f32, isOutput=False)
    qkvw_d = nc.declare_dram_parameter("qkv_wT", [CCH, 128, 3 * C], bf16, isOutput=False)
    qb_d = nc.declare_dram_parameter("qb", [1, C], bf16, isOutput=False)
    kb_d = nc.declare_dram_parameter("kb", [1, C], bf16, isOutput=False)
    vb_d = nc.declare_dram_parameter("vb", [1, C], bf16, isOutput=False)
    pw_d = nc.declare_dram_parameter("proj_wT", [CCH, 128, C], bf16, isOutput=False)
    pb_d = nc.declare_dram_parameter("pb", [CCH, 128, 1], f32, isOutput=False)
    gnw_d = nc.declare_dram_parameter("gnw", [128, CCH], f32, isOutput=False)
    gnb_d = nc.declare_dram_parameter("gnb", [128, CCH], f32, isOutput=False)
    mask_d = nc.declare_dram_parameter("gn_mask", [128, 128], f32, isOutput=False)
    out_d = nc.declare_dram_parameter("out", [CCH, 128, N], f32, isOutput=True)

    def r(ap):
        return ap.bitcast(f32r)

    from contextlib import ExitStack

    with (
        nc.allow_low_precision(reason="float32r tiles feed full-speed matmuls"),
        tile.TileContext(nc) as tc,
        ExitStack() as ctx,
    ):
        consts = ctx.enter_context(tc.tile_pool(name="consts", bufs=1))
        xp = ctx.enter_context(tc.tile_pool(name="xp", bufs=1))
        qkp = ctx.enter_context(tc.tile_pool(name="qkp", bufs=1))
        vtp = ctx.enter_context(tc.tile_pool(name="vtp", bufs=1))
        ap_pool = ctx.enter_context(tc.tile_pool(name="ap", bufs=24))
        op_pool = ctx.enter_context(tc.tile_pool(name="op", bufs=1))
        misc = ctx.enter_context(tc.tile_pool(name="misc", bufs=2))
        gnp = ctx.enter_context(tc.tile_pool(name="gnp", bufs=1))
        outp = ctx.enter_context(tc.tile_pool(name="outp", bufs=2))
        ps_small = ctx.enter_context(tc.tile_pool(name="ps_s", bufs=2, space="PSUM"))
        oup = ctx.enter_context(tc.tile_pool(name="oup", bufs=1))
        repp = ctx.enter_context(tc.tile_pool(name="repp", bufs=2))
        dramp = ctx.enter_context(tc.tile_pool(name="dramp", bufs=1, space="DRAM"))
        ps_big = ctx.enter_context(tc.tile_pool(name="ps_b", bufs=2, space="PSUM"))

        # ---- constants ----
        mask_sb = consts.tile([128, 128], f32, tag="mask")
        nc.sync.dma_start(out=mask_sb, in_=mask_d[:, :])
        vb_row = consts.tile([1, C], bf16, tag="vbrow")
        nc.sync.dma_start(out=vb_row, in_=vb_d[:, :])
        qb_row = consts.tile([1, C], bf16, tag="qbrow")
        nc.sync.dma_start(out=qb_row, in_=qb_d[:, :])
        kb_row = consts.tile([1, C], bf16, tag="kbrow")
        nc.sync.dma_start(out=kb_row, in_=kb_d[:, :])
        ones1 = consts.tile([1, 128], bf16, tag="ones1")
        nc.vector.memset(ones1, 1.0)
        ones_row = consts.tile([1, 512], bf16, tag="onesrow")
        nc.vector.memset(ones_row, 1.0)
        eps_sb = consts.tile([128, 1], f32, tag="eps")
        nc.vector.memset(eps_sb, EPS)
        zero_sb = consts.tile([128, 1], f32, tag="zero")
        nc.vector.memset(zero_sb, 0.0)
        pb_sb = []
        for cc in range(CCH):
            t = consts.tile([128, 1], f32, tag=f"pb{cc}")
            nc.sync.dma_start(out=t, in_=pb_d[cc])
            pb_sb.append(t)
        gnw_all = consts.tile([128, CCH], f32, tag="gnw")
        nc.sync.dma_start(out=gnw_all, in_=gnw_d[:, :])
        gnb_all = consts.tile([128, CCH], f32, tag="gnb")
        nc.sync.dma_start(out=gnb_all, in_=gnb_d[:, :])

        # ---- load x, weights ----
        x_sb = []
        for cc in range(CCH):
            t = xp.tile([128, N], f32, tag=f"x{cc}")
            nc.sync.dma_start(out=t, in_=x_d[cc])
            x_sb.append(t)

        with (
            tc.tile_pool(name="wq", bufs=1) as wq_pool,
            tc.tile_pool(name="xn", bufs=1) as xn_pool,
        ):
            qkvw_sb = []
            for cc in range(CCH):
                t = wq_pool.tile([128, 3 * C], bf16, tag=f"qw{cc}")
                nc.sync.dma_start(out=t, in_=qkvw_d[cc])
                qkvw_sb.append(t)

            # ---- GroupNorm (stats batched across the 4 channel chunks) ----
            mv_all = gnp.tile([128, CCH, 2], f32, tag="mv")
            for cc in range(CCH):
                stats = gnp.tile([128, 2, 6], f32, tag=f"st{cc}")
                for sg in range(2):
                    nc.vector.bn_stats(
                        out=stats[:, sg, :], in_=x_sb[cc][:, sg * 512 : (sg + 1) * 512]
                    )
                nc.vector.bn_aggr(out=mv_all[:, cc, :], in_=stats)
            # st2 = [mean_p, var_p + mean_p^2] for all chunks
            st2 = gnp.tile([128, CCH, 2], f32, tag="s2")
            nc.vector.tensor_copy(out=st2[:, :, 0], in_=mv_all[:, :, 0])
            mean_sq = gnp.tile([128, CCH], f32, tag="msq")
            nc.vector.tensor_mul(out=mean_sq, in0=mv_all[:, :, 0], in1=mv_all[:, :, 0])
            nc.vector.tensor_add(out=st2[:, :, 1], in0=mv_all[:, :, 1], in1=mean_sq)
            # group-average across the 64-channel blocks (exact fp32 matmul)
            ps_st = ps_small.tile([128, CCH * 2], f32, tag="ps")
            dep_nop = nc.tensor.nop(hint="dep").ins
            dep_nop.ins = [nc.tensor.lower_ap(mask_sb), nc.tensor.lower_ap(st2[:, :, :])]
            nc.tensor.matmul(
                ps_st,
                lhsT=mask_sb,
                rhs=st2.rearrange("p c two -> p (c two)"),
                start=True,
                stop=True,
            )
            gst_sb = gnp.tile([128, CCH, 2], f32, tag="gstsb")
            nc.vector.tensor_copy(
                out=gst_sb, in_=ps_st.rearrange("p (c two) -> p c two", two=2)
            )
            gst = gst_sb
            gm2 = gnp.tile([128, CCH], f32, tag="g2")
            nc.vector.tensor_mul(out=gm2, in0=gst[:, :, 0], in1=gst[:, :, 0])
            gvar = gnp.tile([128, CCH], f32, tag="gv")
            nc.vector.tensor_sub(out=gvar, in0=gst[:, :, 1], in1=gm2)
            # rstd = exp(-0.5*ln(var+eps)): stays in the same ACT table set as
            # the attention exp (natural_log_exp), avoiding a table switch.
            lnv = gnp.tile([128, CCH], f32, tag="lnv")
            nc.scalar.activation(out=lnv, in_=gvar, func=AF.Ln, bias=eps_sb)
            rstd = gnp.tile([128, CCH], f32, tag="rstd")
            nc.scalar.activation(out=rstd, in_=lnv, func=AF.Exp, scale=-0.5, bias=zero_sb)
            gscale = gnp.tile([128, CCH], f32, tag="gs")
            nc.vector.tensor_mul(out=gscale, in0=rstd, in1=gnw_all)
            t4 = gnp.tile([128, CCH], f32, tag="t4")
            nc.vector.tensor_mul(out=t4, in0=gst[:, :, 0], in1=gscale)
            gbias = gnp.tile([128, CCH], f32, tag="gb")
            nc.vector.tensor_sub(out=gbias, in0=gnb_all, in1=t4)
            xn_sb = []
            for cc in range(CCH):
                xn = xn_pool.tile([128, N], bf16, tag=f"xn{cc}")
                nc.vector.tensor_scalar(
                    out=xn,
                    in0=x_sb[cc],
                    scalar1=gscale[:, cc : cc + 1],
                    scalar2=gbias[:, cc : cc + 1],
                    op0=OP.mult,
                    op1=OP.add,
                )
                xn_sb.append(xn)

            # ---- qkv matmuls ----
            # vT first so attention pair 0 can start as early as possible.
            # vT layout: [m_chunk partitions, head, 65] (64 d + ones column)
            vt_sb = []
            for mi in range(8):
                vt = vtp.tile([128, NH, HD + 1], bf16, tag=f"vt{mi}")
                nc.vector.memset(vt[:, :, HD : HD + 1], 1.0)
                ps = ps_small.tile([128, C], f32, tag="ps")
                for cc in range(CCH):
                    nc.tensor.matmul(
                        ps,
                        lhsT=(xn_sb[cc][:, mi * 128 : (mi + 1) * 128]),
                        rhs=(qkvw_sb[cc][:, 2 * C : 3 * C]),
                        start=(cc == 0),
                        stop=False,
                    )
                nc.tensor.matmul(
                    ps,
                    lhsT=(ones1[0:1, 0:128]),
                    rhs=(vb_row),
                    start=False,
                    stop=True,
                )
                nc.scalar.copy(
                    out=vt[:, :, 0:HD],
                    in_=ps.rearrange("p (h d) -> p h d", h=NH),
                )
                vt_sb.append(vt)

            # q, k standard layout [o_chunk partitions, n free]; bias folded in
            # as a K=1 matmul; k also folds the 1/sqrt(hd) attention scale.
            q_sb = []
            k_sb = []
            for oc in range(CCH):
                for which, brow, dest in (("q", qb_row, q_sb), ("k", kb_row, k_sb)):
                    base = 0 if which == "q" else C
                    t = qkp.tile([128, N], bf16, tag=f"{which}{oc}")
                    for nj in range(2):
                        ps = ps_small.tile([128, 512], f32, tag="ps")
                        for cc in range(CCH):
                            nc.tensor.matmul(
                                ps,
                                lhsT=(qkvw_sb[cc][:, base + oc * 128 : base + (oc + 1) * 128]),
                                rhs=(xn_sb[cc][:, nj * 512 : (nj + 1) * 512]),
                                start=(cc == 0),
                                stop=False,
                            )
                        nc.tensor.matmul(
                            ps,
                            lhsT=(brow[0:1, oc * 128 : (oc + 1) * 128]),
                            rhs=(ones_row),
                            start=False,
                            stop=True,
                        )
                        tslice = t[:, nj * 512 : (nj + 1) * 512]
                        if which == "q":
                            nc.scalar.copy(out=tslice, in_=ps)
                        else:
                            nc.scalar.mul(out=tslice, in_=ps, mul=float(HD**-0.5))
                    dest.append(t)

        # ---- attention, head-pair by head-pair ----
        o_sb = []
        for p in range(CCH):
            o_tile = op_pool.tile([128, N], bf16, tag=f"o{p}")
            o_sb.append(o_tile)
        gath = []
        for b in range(2):
            g_tile = misc.tile([4, N], f32, tag=f"gath{b}")
            gath.append(g_tile)
        ou_sb = []
        for p in range(CCH):  # head pair p = heads (2p, 2p+1); chunk p of q/k
            a_tiles = [[], []]
            for mi in range(8):
                for hh in range(2):
                    ps_e = ps_big.tile([128, 1024], f32, tag="pse")
                    for nj in range(2):
                        nc.tensor.matmul(
                            ps_e[:, nj * 512 : (nj + 1) * 512],
                            lhsT=(
                                k_sb[p][
                                    hh * 64 : (hh + 1) * 64, mi * 128 : (mi + 1) * 128
                                ]
                            ),
                            rhs=(
                                q_sb[p][
                                    hh * 64 : (hh + 1) * 64, nj * 512 : (nj + 1) * 512
                                ]
                            ),
                            start=True,
                            stop=True,
                        )
                    at = ap_pool.tile([128, 1024], bf16, tag="a")
                    nc.scalar.activation(out=at, in_=ps_e, func=AF.Exp, bias=zero_sb)
                    a_tiles[hh].append(at)
            for hh in range(2):
                h = 2 * p + hh
                av = ps_small.tile([128, 1024], f32, tag="ps")
                for nj in range(2):
                    for mi in range(8):
                        nc.tensor.matmul(
                            av[0 : HD + 1, nj * 512 : (nj + 1) * 512],
                            lhsT=vt_sb[mi][:, h, :],
                            rhs=a_tiles[hh][mi][:, nj * 512 : (nj + 1) * 512],
                            start=(mi == 0),
                            stop=(mi == 7),
                        )
                # softmax sums row -> partition-0 stage (engine writes must be
                # 32-aligned), then DMA into its row of the 4-head gather tile
                stg = misc.tile([1, N], f32, tag="stage")
                nc.vector.tensor_copy(out=stg, in_=av[HD : HD + 1, :])
                nc.sync.dma_start(out=gath[p // 2][h % 4 : h % 4 + 1, :], in_=stg)
                ou = oup.tile([64, N], bf16, tag=f"ou{h}")
                nc.vector.tensor_copy(out=ou, in_=av[0:HD, :])
                ou_sb.append(ou)
            if p % 2 == 1:
                b = p // 2
                rec4 = misc.tile([4, N], bf16, tag=f"rec{b}")
                nc.vector.reciprocal(out=rec4, in_=gath[b])
                # SBUF APs forbid zero partition step, so bounce the 4 rows
                # through DRAM where a broadcast read AP is legal.
                rec_dram = dramp.tile([4, N], bf16, tag=f"recd{b}")
                nc.sync.dma_start(out=rec_dram, in_=rec4)
                for hh4 in range(4):
                    h = b * 4 + hh4
                    rep = repp.tile([64, N], bf16, tag="rep")
                    row = rec_dram[hh4 : hh4 + 1, :]
                    bcast = bass.AP(
                        tensor=row.tensor,
                        offset=row.offset,
                        ap=[[0, 64]] + [list(x) for x in row.ap[1:]],
                    )
                    nc.sync.dma_start(out=rep, in_=bcast)
                    nc.vector.tensor_mul(
                        out=o_sb[h // 2][(h % 2) * 64 : (h % 2 + 1) * 64, :],
                        in0=ou_sb[h],
                        in1=rep,
                    )

        # ---- proj + residual ----
        pw_sb = []
        for cc in range(CCH):
            t = consts.tile([128, C], bf16, tag=f"pw{cc}")
            nc.sync.dma_start(out=t, in_=pw_d[cc])
            pw_sb.append(t)
        for oc in range(CCH):
            ot = outp.tile([128, N], f32, tag="ot")
            for nj in range(2):
                ps = ps_small.tile([128, 512], f32, tag="ps")
                for cc in range(CCH):
                    nc.tensor.matmul(
                        ps,
                        lhsT=(pw_sb[cc][:, oc * 128 : (oc + 1) * 128]),
                        rhs=(o_sb[cc][:, nj * 512 : (nj + 1) * 512]),
                        start=(cc == 0),
                        stop=(cc == CCH - 1),
                    )
                nc.vector.scalar_tensor_tensor(
                    out=ot[:, nj * 512 : (nj + 1) * 512],
                    in0=ps,
                    scalar=pb_sb[oc],
                    in1=x_sb[oc][:, nj * 512 : (nj + 1) * 512],
                    op0=OP.add,
                    op1=OP.add,
                )
            nc.sync.dma_start(out=out_d[oc], in_=ot)

    _split_lw_waits(nc)
    return nc


def _split_lw_waits(nc):
    """This walrus build accepts only ONE sync-wait command per engine
    instruction; Tile can attach several. Hoist each excess wait onto its own
    pure sem-wait instruction inserted just before, in queue order."""
    from concourse import mybir

    for blk in nc.m.functions[0].blocks:
        out = []
        for inst in blk.instructions:
            si = getattr(inst, "sync_info", None)
            if (
                si is not None
                and si.on_wait
                and len(si.on_wait) > 1
                and type(inst).__name__ != "InstEventSemaphore"
            ):
                waits = list(si.on_wait)
                for j, w in enumerate(waits[:-1]):
                    sem = mybir.InstEventSemaphore(
                        name=f"{inst.name}_wsplit{j}",
                        engine=inst.engine,
                        ins=[],
                        outs=[],
                        sync_info=mybir.SyncInfo(on_wait=[w], on_update=[]),
                    )
                    out.append(sem)
                inst.sync_info = mybir.SyncInfo(
                    on_wait=waits[-1:], on_update=list(si.on_update or [])
                )
            out.append(inst)
        blk.instructions = out


def kernel(x, gn_w, gn_b, qkv_w, qkv_b, proj_w, proj_b):
    from concourse.bass_utils import run_bass_kernel_spmd

    B = x.shape[0]
    assert B == NCORES
    if "nc" not in _CACHE:
        _CACHE["nc"] = _build_program()
    nc = _CACHE["nc"]

    xf = np.ascontiguousarray(x.reshape(B, CCH, 128, N).astype(np.float32))
    qkv_wT = np.ascontiguousarray(qkv_w.T.reshape(CCH, 128, 3 * C).astype(ml_dtypes.bfloat16))
    pw_T = np.ascontiguousarray(proj_w.T.reshape(CCH, 128, C).astype(ml_dtypes.bfloat16))
    qb = np.ascontiguousarray(qkv_b[0:C].reshape(1, C).astype(ml_dtypes.bfloat16))
    kb = np.ascontiguousarray(qkv_b[C : 2 * C].reshape(1, C).astype(ml_dtypes.bfloat16))
    vb = np.ascontiguousarray(qkv_b[2 * C : 3 * C].reshape(1, C).astype(ml_dtypes.bfloat16))
    pb = np.ascontiguousarray(proj_b.reshape(CCH, 128, 1).astype(np.float32))
    gnw = np.ascontiguousarray(gn_w.reshape(CCH, 128).T.astype(np.float32))
    gnb = np.ascontiguousarray(gn_b.reshape(CCH, 128).T.astype(np.float32))
    mask = np.zeros((128, 128), dtype=np.float32)
    for g in range(2):
        mask[g * 64 : (g + 1) * 64, g * 64 : (g + 1) * 64] = 1.0 / 64.0

    in_maps = []
    for i in range(NCORES):
        in_maps.append(
            {
                "x": xf[i],
                "qkv_wT": qkv_wT,
                "qb": qb,
                "kb": kb,
                "vb": vb,
                "proj_wT": pw_T,
                "pb": pb,
                "gnw": gnw,
                "gnb": gnb,
                "gn_mask": mask,
            }
        )

    tmpdir = os.environ.get("BASS_TMPDIR")
    if tmpdir:
        os.makedirs(tmpdir, exist_ok=True)
    res = run_bass_kernel_spmd(
        nc,
        in_maps,
        list(range(NCORES)),
        trace=bool(os.environ.get("BASS_TRACE")),
        tmpdir=tmpdir,
    )
    LAST["exec_time_ns"] = res.exec_time_ns
    LAST["results"] = res
    out = np.stack([res.results[i]["out"] for i in range(NCORES)], axis=0)
    return out.reshape(B, C, 32, 32).astype(x.dtype)
